# revision 12
# baseline (speedup 1.0000x reference)
"""Bahdanau attention kernel for Trainium2 (8 NeuronCores, data-parallel over batch).

Reference computation (per batch b):
    pre  = enc[b] @ W_enc + b_enc          # (T, A)   -- b_enc NOTE below
    dec  = hid[b] @ W_dec                  # (A,)
    e    = tanh(pre + dec) @ gvec_w + gvec_b   # (T,)
    e    = where(t >= len_b, -inf, e)
    w    = softmax(2 * e)                  # (T,)
    c    = w @ enc[b]                      # (E,)
    returns (c, w)

Shapes: B=32, T=2048, E=1024, D=1024, A=512.  8 cores x 4 batches.

Device strategy (per core, 4 batches):
  - SWDGE cast-loads enc tiles fp32->fp16 into SBUF ([128t, 1024e] "nat" tiles).
  - PE transpose (matmul-by-identity, fp16 -> PSUM fp16) nat -> encT, DVE copyback.
    (DMA-xbar transpose is unusable: Tile serializes it against all other DMA and
    walrus rejects >1 sync-wait on DMA instructions.)
  - PE: preT[a,t] = W_enc^T chunks @ encT chunks (fp16, fp32 PSUM accum).
  - ACT: th = tanh(preT + dec[a]) fused via activation bias (dec on partitions).
  - PE: e[1,t] = g^T @ th (+ mask row via K=1 matmul; mask holds 0 / -10000).
  - ACT: p = exp(2e) with accum_out giving running sum S.
  - DVE: S -> 1/S, w = p/S (fp16), stored via SWDGE cast to fp32.
  - PE: wT via K=1 matmuls; c = sum_t w_t * enc[t,:] using the resident nat tiles.

gvec_b and b_enc note: softmax is invariant to constant shifts of e, and b_enc
shifts e by a t-independent amount only through... it does NOT: b_enc varies over
a, so it is folded into the tanh bias together with dec (see dec_full below).
gvec_b is a constant shift of e -> cancels in softmax; dropped.
"""

import sys
import numpy as np

try:
    import concourse.bass as bass  # noqa
except ImportError:
    for p in ("/opt/trn_rl_repo", "/root/.axon_site/_ro/trn_rl_repo"):
        if p not in sys.path:
            sys.path.insert(0, p)
    import concourse.bass as bass

import concourse.mybir as mybir
import concourse.tile as tile
from concourse.bass_utils import run_bass_kernel_spmd
from concourse.masks import make_identity

F16 = mybir.dt.float16
F32 = mybir.dt.float32
AF = mybir.ActivationFunctionType

B, T, E, D, A = 32, 2048, 1024, 1024, 512
NCORES = 8
BC = B // NCORES          # batches per core = 4
NT = T // 128             # 16 t-subtiles per batch
NMAC = T // 512           # 4 macro tiles (512 t) per batch
KE = E // 128             # 8 e-chunks
KA = A // 128             # 4 a-chunks
KD = D // 128             # 8 d-chunks
MASK_NEG = -10000.0


def build_nc():
    nc = bass.Bass("TRN2", target_bir_lowering=True, debug=False)

    enc = nc.dram_tensor("enc", [BC, T, E], F32, kind="ExternalInput").ap()
    hidT = nc.dram_tensor("hidT", [D, BC], F32, kind="ExternalInput").ap()
    w_enc = nc.dram_tensor("w_enc", [E, A], F32, kind="ExternalInput").ap()
    w_dec = nc.dram_tensor("w_dec", [D, A], F32, kind="ExternalInput").ap()
    gvec = nc.dram_tensor("gvec", [A, 1], F32, kind="ExternalInput").ap()
    # host-prepared: b_enc broadcast row (A,) as fp32
    benc = nc.dram_tensor("benc", [A], F32, kind="ExternalInput").ap()
    # host-prepared mask: 0.0 valid / -10000.0 masked, fp16 [BC, T]
    mask = nc.dram_tensor("mask", [BC, T], F16, kind="ExternalInput").ap()

    c_out = nc.dram_tensor("c_out", [BC, E], F32, kind="ExternalOutput").ap()
    w_out = nc.dram_tensor("w_out", [BC, T], F32, kind="ExternalOutput").ap()

    with tile.TileContext(nc) as tc:
        _body(tc, enc, hidT, w_enc, w_dec, gvec, benc, mask, c_out, w_out)
    _split_multiwaits(nc)
    return nc


def _split_multiwaits(nc):
    """walrus codegen templates accept at most one sync-wait per real
    instruction; hoist extra waits onto injected NoOps on the same engine."""
    keep = {"InstNoOp"}
    for bb in nc.m.functions[0].blocks:
        il = bb.instructions
        i = 0
        while i < len(il):
            inst = il[i]
            si = getattr(inst, "sync_info", None)
            if (
                si is not None
                and len(si.on_wait) > 1
                and type(inst).__name__ not in keep
            ):
                waits = list(si.on_wait)
                for k, wt in enumerate(waits[:-1]):
                    nop = mybir.InstNoOp(name=f"{inst.name}-hw{k}", ins=[], outs=[])
                    nop.engine = inst.engine
                    nop.sync_info = mybir.SyncInfo(on_wait=[wt], on_update=[])
                    il.insert(i, nop)
                    i += 1
                inst.sync_info = mybir.SyncInfo(
                    on_wait=[waits[-1]], on_update=list(si.on_update)
                )
            i += 1
    return nc


def _body(tc, enc, hidT, w_enc, w_dec, gvec, benc, mask, c_out, w_out):
    nc = tc.nc
    from contextlib import ExitStack

    with ExitStack() as ctx:
        const = ctx.enter_context(tc.tile_pool(name="const", bufs=1))
        natp = ctx.enter_context(tc.tile_pool(name="nat", bufs=2))
        tpp = ctx.enter_context(tc.tile_pool(name="tp", bufs=2))
        thp = ctx.enter_context(tc.tile_pool(name="th", bufs=2))
        pbp = ctx.enter_context(tc.tile_pool(name="pb", bufs=2))
        ps_tp = ctx.enter_context(tc.tile_pool(name="ps_tp", bufs=2, space="PSUM"))
        ps_pre = ctx.enter_context(tc.tile_pool(name="ps_pre", bufs=2, space="PSUM"))
        ps_e = ctx.enter_context(tc.tile_pool(name="ps_e", bufs=1, space="PSUM"))
        ps_wt = ctx.enter_context(tc.tile_pool(name="ps_wt", bufs=1, space="PSUM"))
        ps_c = ctx.enter_context(tc.tile_pool(name="ps_c", bufs=2, space="PSUM"))

        # ---- constants ----
        w16 = const.tile([128, KE, A], F16)      # W_enc[e,a] -> [p, ke, a]
        nc.gpsimd.dma_start(w16[:], w_enc.rearrange("(ke p) a -> p ke a", p=128))
        wd16 = const.tile([128, KD, A], F16)
        nc.gpsimd.dma_start(wd16[:], w_dec.rearrange("(kd p) a -> p kd a", p=128))
        hidT16 = const.tile([128, KD, BC], F16)
        nc.gpsimd.dma_start(hidT16[:], hidT.rearrange("(kd p) b -> p kd b", p=128))
        g16 = const.tile([128, KA], F16)
        nc.gpsimd.dma_start(g16[:], gvec[:, 0].rearrange("(ka p) -> p ka", p=128))
        bencS = const.tile([128, KA], F32)       # b_enc[a] -> [p, ka]
        nc.gpsimd.dma_start(bencS[:], benc.rearrange("(ka p) -> p ka", p=128))
        mask16 = const.tile([1, BC, T], F16)
        nc.gpsimd.dma_start(mask16[:], mask.rearrange("b t -> (b t)"))
        ones16 = const.tile([1, 1], F16)
        nc.vector.memset(ones16[:], 1.0)
        ident16 = const.tile([128, 128], F16)
        make_identity(nc, ident16[:])

        # ---- dec = hid @ W_dec, stored transposed+biased:
        # decfull[:, ka*BC + b] = dec[b, ka*128 + p] + b_enc[ka*128 + p]
        decT = const.tile([128, KA * BC], F32)
        for ka in range(KA):
            psd = ps_e.tile([128, BC], F32, tag="e")
            for kd in range(KD):
                nc.tensor.matmul(
                    psd[:],
                    wd16[:, kd, 128 * ka:128 * (ka + 1)],
                    hidT16[:, kd, :],
                    start=(kd == 0),
                    stop=(kd == KD - 1),
                )
            # add b_enc chunk while copying out (bias is per-partition [128,1])
            nc.scalar.activation(
                decT[:, ka * BC:(ka + 1) * BC], psd[:], AF.Identity,
                bias=bencS[:, ka:ka + 1], scale=1.0,
            )

        # ---- per-batch state allocated lazily ----
        state = {}

        def emit_macro(b, m, nat_b):
            t0 = m * 512
            # loads: 2 SWDGE cast-load calls of 2 subtiles each
            for half in range(2):
                jj = 4 * m + 2 * half
                nc.gpsimd.dma_start(
                    nat_b[:, jj:jj + 2, :],
                    enc[b, t0 + 256 * half: t0 + 256 * (half + 1), :]
                    .rearrange("(n p) e -> p n e", p=128),
                )
            # PE transposes (fp16 matmul-by-identity into a PSUM bank), DVE copyback
            encT = tpp.tile([128, 4, KE, 128], F16, tag="encT")
            for j in range(4):
                tp_ps = ps_tp.tile([128, KE, 128], F16, tag="tp")
                for k in range(KE):
                    nc.tensor.transpose(
                        tp_ps[:, k], nat_b[:, 4 * m + j, 128 * k:128 * (k + 1)],
                        ident16[:],
                    )
                nc.vector.tensor_copy(encT[:, j], tp_ps[:])
            # e-dot accumulator: mask row first (K=1), then per-ka g-dots (K=128)
            e_ps = ps_e.tile([1, 512], F32, tag="e")
            nc.tensor.matmul(
                e_ps[:], ones16[:], mask16[:, b, t0:t0 + 512],
                start=True, stop=False,
            )
            th = thp.tile([128, KA, 512], F16, tag="th")
            for ka in range(KA):
                # preT[a,t] accumulation over e-chunks
                pst = ps_pre.tile([128, 512], F32, tag="pre")
                for ke in range(KE):
                    nc.tensor.matmul(
                        pst[:],
                        w16[:, ke, 128 * ka:128 * (ka + 1)],
                        encT[:, :, ke, :],
                        start=(ke == 0),
                        stop=(ke == KE - 1),
                    )
                # tanh (fused +dec +b_enc bias), cast to fp16
                nc.scalar.activation(
                    th[:, ka], pst[:], AF.Tanh,
                    bias=decT[:, ka * BC + b: ka * BC + b + 1], scale=1.0,
                )
                nc.tensor.matmul(
                    e_ps[:], g16[:, ka:ka + 1], th[:, ka],
                    start=False, stop=(ka == KA - 1),
                )
            # exp(2e), accumulate S chunk
            p_b, Ssum = state[b]["p"], state[b]["Ssum"]
            nc.scalar.activation(
                p_b[:, t0:t0 + 512], e_ps[:], AF.Exp,
                bias=0.0, scale=2.0, accum_out=Ssum[:, m:m + 1],
            )

        def emit_batch_head(b):
            state[b] = {
                "p": pbp.tile([1, T], F32, tag="p", name=f"p_{b}"),
                "Ssum": pbp.tile([1, NMAC], F32, tag="Ssum", name=f"Ssum_{b}"),
                "w16": pbp.tile([1, T], F16, tag="w16", name=f"w16_{b}"),
                "S": pbp.tile([1, 1], F32, tag="S", name=f"S_{b}"),
                "rS": pbp.tile([1, 1], F32, tag="rS", name=f"rS_{b}"),
            }

        def emit_batch_tail(b):
            st = state[b]
            nc.vector.tensor_reduce(
                st["S"][:], st["Ssum"][:], mybir.AxisListType.X, mybir.AluOpType.add
            )
            nc.vector.reciprocal(st["rS"][:], st["S"][:])
            nc.vector.tensor_scalar_mul(st["w16"][:], st["p"][:], st["rS"][:])
            # w output (fp16 -> fp32 cast on SWDGE)
            nc.gpsimd.dma_start(w_out[b:b + 1, :], st["w16"][:])

        def emit_phase2(b, nat_b):
            st = state[b]
            wt_ps = ps_wt.tile([128, NT], F32, tag="wt")
            for j in range(NT):
                nc.tensor.matmul(
                    wt_ps[:, j:j + 1], st["w16"][:, 128 * j:128 * (j + 1)],
                    ones16[:], start=True, stop=True,
                )
            wt16 = pbp.tile([128, NT], F16, tag="wt16")
            nc.scalar.activation(wt16[:], wt_ps[:], AF.Copy)
            c_sb = pbp.tile([1, E], F32, tag="c_sb")
            for h in range(2):
                c_ps = ps_c.tile([1, 512], F32, tag="c")
                for j in range(NT):
                    nc.tensor.matmul(
                        c_ps[:], wt16[:, j:j + 1],
                        nat_b[:, j, 512 * h:512 * (h + 1)],
                        start=(j == 0), stop=(j == NT - 1),
                    )
                nc.scalar.activation(c_sb[:, 512 * h:512 * (h + 1)], c_ps[:], AF.Copy)
            nc.gpsimd.dma_start(c_out[b:b + 1, :], c_sb[:])

        pending = None  # (batch, nat tile) whose phase2 is deferred
        for b in range(BC):
            nat_b = natp.tile([128, NT, E], F16, tag="nat")
            emit_batch_head(b)
            for m in range(NMAC):
                emit_macro(b, m, nat_b)
                if pending is not None and m == 1:
                    emit_phase2(*pending)
                    pending = None
            emit_batch_tail(b)
            pending = (b, nat_b)
        emit_phase2(*pending)


_NC_CACHE = None


def kernel(encoder_outputs, encoder_output_lens, decoder_hidden_state,
           W_enc, b_enc, W_dec, gvec_w, gvec_b, scaling=2.0):
    global _NC_CACHE
    if _NC_CACHE is None:
        _NC_CACHE = build_nc()
    nc = _NC_CACHE

    enc = np.ascontiguousarray(np.asarray(encoder_outputs, dtype=np.float32))
    lens = np.maximum(np.asarray(encoder_output_lens).astype(np.int64), 1)
    hid = np.asarray(decoder_hidden_state, dtype=np.float32)
    W_enc = np.ascontiguousarray(np.asarray(W_enc, dtype=np.float32))
    W_dec = np.ascontiguousarray(np.asarray(W_dec, dtype=np.float32))
    gvec_w = np.ascontiguousarray(np.asarray(gvec_w, dtype=np.float32))
    b_enc = np.ascontiguousarray(np.asarray(b_enc, dtype=np.float32))

    mask_full = np.where(
        np.arange(T)[None, :] < lens[:, None], 0.0, MASK_NEG
    ).astype(np.float16)
    hidT_full = np.ascontiguousarray(hid.T)  # [D, B]

    in_maps = []
    for i in range(NCORES):
        sl = slice(BC * i, BC * (i + 1))
        in_maps.append({
            "enc": enc[sl],
            "hidT": np.ascontiguousarray(hidT_full[:, sl]),
            "w_enc": W_enc,
            "w_dec": W_dec,
            "gvec": gvec_w,
            "benc": b_enc,
            "mask": np.ascontiguousarray(mask_full[sl]),
        })

    res = run_bass_kernel_spmd(nc, in_maps, list(range(NCORES)))
    c = np.concatenate([res.results[i]["c_out"] for i in range(NCORES)], axis=0)
    w = np.concatenate([res.results[i]["w_out"] for i in range(NCORES)], axis=0)
    return (c, w)
